# revision 2
# baseline (speedup 1.0000x reference)
"""CrossModalAttention Trainium2 kernel (8-core data parallel), V2.

Math: with seq_len=1, softmax over one key == 1, so each MultiheadAttention
collapses to   att = (kv @ Wv.T + bv) @ Wo.T + bo = kv @ Wc.T + bc
with Wc = Wo @ Wv (256x256) and bc = bv @ Wo.T + bo, followed by
    out = LayerNorm(x + att) * g + b.

V2 dataflow (per core, 16384 rows per modality, bf16 I/O):
  - Host ships activations TRANSPOSED (feat on partitions) in bf16.
  - Matmul runs with the x.T block as the STATIONARY operand and the
    256-wide weight as the moving operand, so `att` lands in PSUM in
    NATURAL layout (rows on partitions) - no output transpose pass.
  - The residual is accumulated into the same PSUM group by two extra
    identity-moving matmuls (out += x_mod.T_chunk.T @ I), so no
    element-wise residual pass on DVE.
  - Per 128-row block: one bn_stats (DVE) from PSUM, even/odd stat
    halves combined with tensor_tensor ops on the otherwise-idle
    GPSIMD/Pool engine, sqrt on ACT, reciprocal on DVE.
  - Normalize: one ACT pass per block (scale=rstd, bias=-mean*rstd),
    writing bf16 natural-layout output tiles; DMA out 1KB rows
    (both modalities interleaved per row).
"""

import os
import numpy as np

N_CORES = 8
B = 131072
E = 256
EPS = 1e-5
ROWS = B // N_CORES          # rows per core per modality
SUPER = 1024                 # rows per DMA super-tile
N_SUPER = ROWS // SUPER
NB = SUPER // 128            # 128-row blocks per (super, mod) job

_PROGRAM_CACHE = {}


def _build_program(generic_gb, generic_bc):
    import concourse.bass as bass
    import concourse.tile as tile
    from concourse import bacc, mybir
    from concourse.masks import make_identity

    f32 = mybir.dt.float32
    bf16 = mybir.dt.bfloat16
    AF = mybir.ActivationFunctionType
    OP = mybir.AluOpType

    nc = bacc.Bacc("TRN2")

    # ---- DRAM I/O ----
    xT = nc.dram_tensor("xT", [2, E, ROWS], bf16, kind="ExternalInput")
    # w[mod] = Wc[mod].T laid out (feat_in, feat_out); mod 0 produces the
    # audio output (source = text), mod 1 the text output (source = audio).
    w = nc.dram_tensor("w", [2, E, E], bf16, kind="ExternalInput")
    if generic_bc:
        bc = nc.dram_tensor("bc", [2, 1, E], bf16, kind="ExternalInput")
    if generic_gb:
        g = nc.dram_tensor("g", [2, 1, E], f32, kind="ExternalInput")
        b = nc.dram_tensor("b", [2, 1, E], f32, kind="ExternalInput")
    # y rows interleave the two modalities: [row, mod, feat] -> 1KB DMA lines
    y = nc.dram_tensor("y", [ROWS, 2, E], bf16, kind="ExternalOutput")

    xT_v = xT.rearrange("m (c p) n -> p m c n", p=128)
    w_v = w.rearrange("m (k p) n -> p m k n", p=128)
    y_v = y.rearrange("(t p) m d -> p t m d", p=128)

    with tile.TileContext(nc) as tc:
        with (
            tc.tile_pool(name="const", bufs=1) as const_pool,
            tc.tile_pool(name="xin", bufs=4) as xin_pool,
            tc.tile_pool(name="yout", bufs=3) as yout_pool,
            tc.tile_pool(name="stats", bufs=4) as stats_pool,
            tc.tile_pool(name="zps", bufs=2, space="PSUM") as zps_pool,
        ):
            # ---- constants ----
            w_sb = const_pool.tile([128, 2, 2, E], bf16)  # [p, mod, k, n]
            nc.sync.dma_start(out=w_sb, in_=w_v)
            ident = const_pool.tile([128, 128], bf16)
            make_identity(nc, ident)
            half_sb = const_pool.tile([128, NB], f32)
            nc.vector.memset(half_sb, 0.5)
            invE_sb = const_pool.tile([128, NB], f32)
            nc.vector.memset(invE_sb, 1.0 / E)
            epsb_sb = const_pool.tile([128, NB], f32)
            nc.vector.memset(epsb_sb, EPS)
            nhalf_sb = const_pool.tile([128, NB], f32)
            nc.vector.memset(nhalf_sb, -0.5)
            if generic_bc:
                ones1_sb = const_pool.tile([1, 128], bf16)
                nc.vector.memset(ones1_sb, 1.0)
                bc_sb = const_pool.tile([1, 2, E], bf16)
                nc.sync.dma_start(out=bc_sb, in_=bc.rearrange("m o n -> o m n"))
            if generic_gb:
                gb_sb = const_pool.tile([128, 2, 2, E], f32)
                for mod in range(2):
                    nc.sync.dma_start(
                        out=gb_sb[:, mod, 0], in_=g[mod].to_broadcast((128, E))
                    )
                    nc.sync.dma_start(
                        out=gb_sb[:, mod, 1], in_=b[mod].to_broadcast((128, E))
                    )

            for sp in range(N_SUPER):
                # ---- load super-tile (both modalities, transposed bf16) ----
                xT_sb = xin_pool.tile([128, 2, 2, SUPER], bf16, tag="xin")
                nc.sync.dma_start(
                    out=xT_sb,
                    in_=xT_v[:, :, :, sp * SUPER:(sp + 1) * SUPER],
                )
                y_sb = yout_pool.tile([128, NB, 2, E], bf16, tag="yout")

                for mod in range(2):
                    src = 1 - mod
                    # ---- z = x_mod + x_src @ Wc.T  (natural layout) ----
                    z_ps = zps_pool.tile([128, NB, E], f32, tag="z")
                    for blk in range(NB):
                        r0 = blk * 128
                        for k in range(2):
                            nc.tensor.matmul(
                                z_ps[:, blk, :],
                                xT_sb[:, src, k, r0:r0 + 128],
                                w_sb[:, mod, k, :],
                                start=(k == 0),
                                stop=False,
                                skip_group_check=True,
                            )
                        if generic_bc:
                            nc.tensor.matmul(
                                z_ps[:, blk, :],
                                ones1_sb,
                                bc_sb[:, mod, :],
                                start=False,
                                stop=False,
                                skip_group_check=True,
                            )
                        for c in range(2):
                            nc.tensor.matmul(
                                z_ps[:, blk, c * 128:(c + 1) * 128],
                                xT_sb[:, mod, c, r0:r0 + 128],
                                ident,
                                start=False,
                                stop=(c == 1),
                                skip_group_check=True,
                            )
                    # ---- stats: one bn_stats per block (HW limit) ----
                    st = stats_pool.tile([128, NB, 6], f32, tag="st")
                    for blk in range(NB):
                        nc.vector.bn_stats(
                            out=st[:, blk, :], in_=z_ps[:, blk, :],
                        )
                    # combine even/odd halves on the Pool engine:
                    #   mean = (m_e + m_o)/2
                    #   var  = (cv_e + cv_o)/256 + ((m_e - m_o)/2)^2
                    m_e, cv_e = st[:, :, 1], st[:, :, 2]
                    m_o, cv_o = st[:, :, 4], st[:, :, 5]
                    tmp = stats_pool.tile([128, 8, NB], f32, tag="tmp")
                    a, d, cv, hd, dd, cvs, vs, varp = (
                        tmp[:, i, :] for i in range(8)
                    )
                    nc.gpsimd.tensor_add(a, m_e, m_o)
                    nc.gpsimd.tensor_sub(d, m_e, m_o)
                    nc.gpsimd.tensor_add(cv, cv_e, cv_o)
                    nc.gpsimd.tensor_mul(hd, d, half_sb)
                    nc.gpsimd.tensor_mul(dd, hd, hd)
                    nc.gpsimd.tensor_mul(cvs, cv, invE_sb)
                    nc.gpsimd.tensor_add(vs, cvs, dd)
                    nc.gpsimd.tensor_add(varp, vs, epsb_sb)
                    # rstd = 1/sqrt(varp); nmrs = -mean*rstd
                    sdr = stats_pool.tile([128, 3, NB], f32, tag="sdr")
                    sd, rstd, nmrs = (sdr[:, i, :] for i in range(3))
                    nc.scalar.activation(
                        out=sd, in_=varp, func=AF.Sqrt, scale=1.0,
                    )
                    nc.vector.reciprocal(out=rstd, in_=sd)
                    nc.vector.scalar_tensor_tensor(
                        out=nmrs, in0=a, scalar=-0.5,
                        in1=rstd, op0=OP.mult, op1=OP.mult,
                    )
                    # ---- normalize: y = (z - m) * rstd  (ACT per block) ----
                    for blk in range(NB):
                        nc.scalar.activation(
                            out=y_sb[:, blk, mod, :],
                            in_=z_ps[:, blk, :],
                            func=AF.Identity,
                            bias=nmrs[:, blk:blk + 1],
                            scale=rstd[:, blk:blk + 1],
                        )
                        if generic_gb:
                            nc.vector.tensor_mul(
                                y_sb[:, blk, mod, :],
                                y_sb[:, blk, mod, :],
                                gb_sb[:, mod, 0],
                            )
                            nc.vector.tensor_add(
                                y_sb[:, blk, mod, :],
                                y_sb[:, blk, mod, :],
                                gb_sb[:, mod, 1],
                            )

                # ---- store super-tile ----
                t0 = sp * NB
                nc.sync.dma_start(
                    out=y_v[:, t0:t0 + NB, :, :],
                    in_=y_sb,
                )

    nc.finalize()
    return nc


def _get_program(generic_gb, generic_bc):
    key = (bool(generic_gb), bool(generic_bc))
    if key not in _PROGRAM_CACHE:
        _PROGRAM_CACHE[key] = _build_program(*key)
    return _PROGRAM_CACHE[key]


def _prep_host(audio_embed, text_embed,
               a2t_in_w, a2t_in_b, a2t_out_w, a2t_out_b,
               t2a_in_w, t2a_in_b, t2a_out_w, t2a_out_b,
               ln1_g, ln1_b, ln2_g, ln2_b):
    import ml_dtypes
    f = np.float32
    bf = ml_dtypes.bfloat16
    # fold the two projections: att = kv @ (Wo @ Wv).T + (bv @ Wo.T + bo)
    wv_a, bv_a = a2t_in_w[2 * E:], a2t_in_b[2 * E:]
    wv_t, bv_t = t2a_in_w[2 * E:], t2a_in_b[2 * E:]
    wc_a = (np.asarray(a2t_out_w, np.float64) @ np.asarray(wv_a, np.float64)).astype(f)
    wc_t = (np.asarray(t2a_out_w, np.float64) @ np.asarray(wv_t, np.float64)).astype(f)
    bc_a = (np.asarray(bv_a, np.float64) @ np.asarray(a2t_out_w, np.float64).T
            + np.asarray(a2t_out_b, np.float64)).astype(f)
    bc_t = (np.asarray(bv_t, np.float64) @ np.asarray(t2a_out_w, np.float64).T
            + np.asarray(t2a_out_b, np.float64)).astype(f)

    generic_gb = not (
        np.all(np.asarray(ln1_g) == 1.0) and np.all(np.asarray(ln1_b) == 0.0)
        and np.all(np.asarray(ln2_g) == 1.0) and np.all(np.asarray(ln2_b) == 0.0)
    )
    generic_bc = not (np.all(bc_a == 0.0) and np.all(bc_t == 0.0))

    audio = np.ascontiguousarray(audio_embed, dtype=f)
    text = np.ascontiguousarray(text_embed, dtype=f)

    from concurrent.futures import ThreadPoolExecutor

    def shard_xT(c):
        out = np.empty((2, E, ROWS), bf)
        out[0] = audio[c * ROWS:(c + 1) * ROWS].T.astype(bf)
        out[1] = text[c * ROWS:(c + 1) * ROWS].T.astype(bf)
        return out

    with ThreadPoolExecutor(max_workers=8) as ex:
        xTs = list(ex.map(shard_xT, range(N_CORES)))

    # w[mod] = Wc[mod].T (feat_in, feat_out)
    w_all = np.stack([wc_a.T, wc_t.T]).astype(bf)
    in_maps = []
    for c in range(N_CORES):
        m = {"xT": xTs[c], "w": w_all}
        if generic_bc:
            m["bc"] = np.stack([
                bc_a.reshape(1, E), bc_t.reshape(1, E),
            ]).astype(bf)
        if generic_gb:
            m["g"] = np.stack([
                np.asarray(ln1_g, f).reshape(1, E),
                np.asarray(ln2_g, f).reshape(1, E),
            ])
            m["b"] = np.stack([
                np.asarray(ln1_b, f).reshape(1, E),
                np.asarray(ln2_b, f).reshape(1, E),
            ])
        in_maps.append(m)
    return in_maps, generic_gb, generic_bc


def _run(in_maps, generic_gb, generic_bc, trace=False):
    import sys
    if "/opt/trn_rl_repo" not in sys.path:
        sys.path.insert(0, "/opt/trn_rl_repo")
    from concourse.bass_utils import run_bass_kernel_spmd

    nc = _get_program(generic_gb, generic_bc)
    res = run_bass_kernel_spmd(
        nc, in_maps, list(range(N_CORES)), trace=trace,
    )
    return res


def kernel(**inputs):
    import sys
    if "/opt/trn_rl_repo" not in sys.path:
        sys.path.insert(0, "/opt/trn_rl_repo")
    in_maps, generic_gb, generic_bc = _prep_host(**inputs)
    res = _run(in_maps, generic_gb, generic_bc,
               trace=bool(os.environ.get("KERNEL_TRACE")))
    f = np.float32
    audio_out = np.concatenate(
        [r["y"][:, 0, :].astype(f) for r in res.results], axis=0)
    text_out = np.concatenate(
        [r["y"][:, 1, :].astype(f) for r in res.results], axis=0)
    kernel.last_exec_time_ns = res.exec_time_ns
    kernel.last_results = res
    return (audio_out, text_out)


# revision 8
# speedup vs baseline: 1.9603x; 1.9603x over previous
"""CrossModalAttention Trainium2 kernel (8-core data parallel), V3.

Math: with seq_len=1, softmax over one key == 1, so each MultiheadAttention
collapses to   att = (kv @ Wv.T + bv) @ Wo.T + bo = kv @ Wc.T + bc
with Wc = Wo @ Wv (256x256) and bc = bv @ Wo.T + bo, followed by
    out = LayerNorm(x + att) * g + b.

V3 dataflow (per core, 16384 rows per modality, bf16 I/O):
  - Host ships activations TRANSPOSED (feat on partitions) in bf16 with a
    row-pair permutation: device column t*256 + s*128 + p holds natural row
    256*t + 2*p + s.  Partition p therefore carries the row PAIR
    (2p, 2p+1) of each 256-row tile, split across the two parities s.
  - Matmuls run with the x.T block as STATIONARY and the 256-wide weight
    moving, so `att` lands in PSUM in natural layout; the residual is
    accumulated into the same PSUM group by identity-moving matmuls.
  - bn_stats reads each 256-row tile through an interleaving access
    pattern (feat-major, parity-minor), so its even/odd statistics split
    yields EXACT per-row stats for both parities in ONE instruction —
    no bn_aggr / combine chain at all.
  - sqrt on ACT, reciprocal + (-mean*rstd) on DVE (tiny batched ops).
  - Normalize: ACT pass per (tile, parity) with per-partition scale/bias
    (one per job moved to DVE tensor_scalar for engine balance), writing
    bf16 natural-layout rows; DMA out 2KB lines (row pair x 2 modalities).
"""

import os
import numpy as np

N_CORES = 8
B = 131072
E = 256
EPS = 1e-5
ROWS = B // N_CORES          # rows per core per modality
SUPER = 1024                 # rows per DMA super-tile / job
N_SUPER = ROWS // SUPER
NT = 2                       # 256-row tiles per 512-row job
JOBS_PER_SUPER = SUPER // (NT * 256)
NORM_DVE = {3}               # (t*2+s) indices normalized on DVE, rest ACT

_PROGRAM_CACHE = {}


def _build_program(generic_gb, generic_bc):
    import concourse.bass as bass
    import concourse.tile as tile
    from concourse import bacc, mybir
    from concourse.masks import make_identity

    f32 = mybir.dt.float32
    bf16 = mybir.dt.bfloat16
    AF = mybir.ActivationFunctionType
    OP = mybir.AluOpType

    nc = bacc.Bacc("TRN2")

    # ---- DRAM I/O ----
    xT = nc.dram_tensor("xT", [2, E, ROWS], bf16, kind="ExternalInput")
    # w[mod] = Wc[mod].T laid out (feat_in, feat_out); mod 0 produces the
    # audio output (source = text), mod 1 the text output (source = audio).
    w = nc.dram_tensor("w", [2, E, E], bf16, kind="ExternalInput")
    if generic_bc:
        bc = nc.dram_tensor("bc", [2, 1, E], bf16, kind="ExternalInput")
    if generic_gb:
        g = nc.dram_tensor("g", [2, 1, E], f32, kind="ExternalInput")
        b = nc.dram_tensor("b", [2, 1, E], f32, kind="ExternalInput")
    # y rows natural order; per (p, tile): row pair x modalities = 2KB lines
    y = nc.dram_tensor("y", [ROWS, 2, E], bf16, kind="ExternalOutput")

    xT_v = xT.rearrange("m (c p) n -> p m c n", p=128)
    w_v = w.rearrange("m (k p) n -> p m k n", p=128)
    y_v = y.rearrange("(t p s) m d -> p t (s m d)", p=128, s=2)

    with tile.TileContext(nc) as tc:
        with (
            tc.tile_pool(name="const", bufs=1) as const_pool,
            tc.tile_pool(name="xin", bufs=4) as xin_pool,
            tc.tile_pool(name="yout", bufs=3) as yout_pool,
            tc.tile_pool(name="stats", bufs=6) as stats_pool,
            tc.tile_pool(name="zps", bufs=4, space="PSUM") as zps_pool,
        ):
            # ---- constants ----
            w_sb = const_pool.tile([128, 2, 2, E], bf16)  # [p, mod, k, n]
            nc.sync.dma_start(out=w_sb, in_=w_v)
            ident = const_pool.tile([128, 128], bf16)
            make_identity(nc, ident)
            eps_sb = const_pool.tile([128, 1], f32)
            nc.vector.memset(eps_sb, EPS)
            if generic_bc:
                ones1_sb = const_pool.tile([1, 128], bf16)
                nc.vector.memset(ones1_sb, 1.0)
                bc_sb = const_pool.tile([1, 2, E], bf16)
                nc.sync.dma_start(out=bc_sb, in_=bc.rearrange("m o n -> o m n"))
            if generic_gb:
                gb_sb = const_pool.tile([128, 2, 2, E], f32)
                for mod in range(2):
                    nc.sync.dma_start(
                        out=gb_sb[:, mod, 0], in_=g[mod].to_broadcast((128, E))
                    )
                    nc.sync.dma_start(
                        out=gb_sb[:, mod, 1], in_=b[mod].to_broadcast((128, E))
                    )

            for sp in range(N_SUPER):
                # ---- load super-tile (both modalities, transposed bf16) ----
                xT_sb = xin_pool.tile([128, 2, 2, SUPER], bf16, tag="xin")
                nc.sync.dma_start(
                    out=xT_sb,
                    in_=xT_v[:, :, :, sp * SUPER:(sp + 1) * SUPER],
                )
                ST = SUPER // 256    # 256-row tiles per super
                y_sb = yout_pool.tile([128, ST, 2, 2, E], bf16, tag="yout")

                for jb in range(JOBS_PER_SUPER):
                  for mod in range(2):
                    src = 1 - mod
                    # ---- z = x_mod + x_src @ Wc.T  (natural layout) ----
                    z_ps = zps_pool.tile([128, NT, 2, E], f32, tag="z")
                    for t in range(NT):
                        for s in range(2):
                            c0 = jb * NT * 256 + t * 256 + s * 128
                            for k in range(2):
                                nc.tensor.matmul(
                                    z_ps[:, t, s, :],
                                    xT_sb[:, src, k, c0:c0 + 128],
                                    w_sb[:, mod, k, :],
                                    start=(k == 0),
                                    stop=False,
                                    skip_group_check=True,
                                )
                            if generic_bc:
                                nc.tensor.matmul(
                                    z_ps[:, t, s, :],
                                    ones1_sb,
                                    bc_sb[:, mod, :],
                                    start=False,
                                    stop=False,
                                    skip_group_check=True,
                                )
                            for c in range(2):
                                nc.tensor.matmul(
                                    z_ps[:, t, s, c * 128:(c + 1) * 128],
                                    xT_sb[:, mod, c, c0:c0 + 128],
                                    ident,
                                    start=False,
                                    stop=(c == 1),
                                    skip_group_check=True,
                                )
                    # ---- stats: one bn_stats per 256-row tile.  The
                    # interleaving read (feat-major, parity-minor) makes the
                    # even/odd split give exact per-row stats per parity.
                    st = stats_pool.tile([128, NT, 6], f32, tag="st")
                    for t in range(NT):
                        # bn_stats over the feat-major/parity-minor stream;
                        # bypass the builder's shape check (it would read the
                        # 3D view as 256 groups of 2 - HW consumes the AP as
                        # a flat positional stream of 512).
                        ve = nc.vector
                        ve.add_instruction(
                            mybir.InstBNStats(
                                name=ve.bass.get_next_instruction_name(),
                                ins=[ve.lower_ap(
                                    z_ps[:, t].rearrange("p s d -> p d s")
                                )],
                                outs=[ve.lower_ap(st[:, t, :])],
                            )
                        )
                    # records: [count, mean, count*var] x (parity0, parity1)
                    means = st.rearrange("p t (r v) -> p (t r) v", r=2)[:, :, 1]
                    cvs = st.rearrange("p t (r v) -> p (t r) v", r=2)[:, :, 2]
                    # rstd = 1/sqrt(cv/256 + eps); nmrs = -mean * rstd
                    srn = stats_pool.tile([128, 3, NT * 2], f32, tag="srn")
                    sd, rstd, nmrs = (srn[:, i, :] for i in range(3))
                    nc.scalar.activation(
                        out=sd, in_=cvs, func=AF.Sqrt,
                        bias=eps_sb, scale=1.0 / E,
                    )
                    nc.vector.reciprocal(out=rstd, in_=sd)
                    nc.vector.scalar_tensor_tensor(
                        out=nmrs, in0=means, scalar=-1.0,
                        in1=rstd, op0=OP.mult, op1=OP.mult,
                    )
                    # ---- normalize: y = (z - m) * rstd ----
                    for t in range(NT):
                        for s in range(2):
                            i = t * 2 + s
                            tt = jb * NT + t
                            if i in NORM_DVE:
                                nc.vector.tensor_scalar(
                                    out=y_sb[:, tt, s, mod, :],
                                    in0=z_ps[:, t, s, :],
                                    scalar1=rstd[:, i:i + 1],
                                    scalar2=nmrs[:, i:i + 1],
                                    op0=OP.mult,
                                    op1=OP.add,
                                )
                            else:
                                nc.scalar.activation(
                                    out=y_sb[:, tt, s, mod, :],
                                    in_=z_ps[:, t, s, :],
                                    func=AF.Identity,
                                    bias=nmrs[:, i:i + 1],
                                    scale=rstd[:, i:i + 1],
                                )
                            if generic_gb:
                                nc.vector.tensor_mul(
                                    y_sb[:, tt, s, mod, :],
                                    y_sb[:, tt, s, mod, :],
                                    gb_sb[:, mod, 0],
                                )
                                nc.vector.tensor_add(
                                    y_sb[:, tt, s, mod, :],
                                    y_sb[:, tt, s, mod, :],
                                    gb_sb[:, mod, 1],
                                )

                # ---- store super-tile ----
                t0 = sp * ST
                nc.sync.dma_start(
                    out=y_v[:, t0:t0 + ST, :],
                    in_=y_sb.rearrange("p t s m d -> p t (s m d)"),
                )

    nc.finalize()
    return nc


def _get_program(generic_gb, generic_bc):
    key = (bool(generic_gb), bool(generic_bc))
    if key not in _PROGRAM_CACHE:
        _PROGRAM_CACHE[key] = _build_program(*key)
    return _PROGRAM_CACHE[key]


def _make_perm():
    # device column t*256 + s*128 + p  <->  natural row 256*t + 2*p + s
    col = np.arange(ROWS)
    t, rem = col // 256, col % 256
    s, p = rem // 128, rem % 128
    return 256 * t + 2 * p + s


def _prep_host(audio_embed, text_embed,
               a2t_in_w, a2t_in_b, a2t_out_w, a2t_out_b,
               t2a_in_w, t2a_in_b, t2a_out_w, t2a_out_b,
               ln1_g, ln1_b, ln2_g, ln2_b):
    import ml_dtypes
    f = np.float32
    bf = ml_dtypes.bfloat16
    # fold the two projections: att = kv @ (Wo @ Wv).T + (bv @ Wo.T + bo)
    wv_a, bv_a = a2t_in_w[2 * E:], a2t_in_b[2 * E:]
    wv_t, bv_t = t2a_in_w[2 * E:], t2a_in_b[2 * E:]
    wc_a = (np.asarray(a2t_out_w, np.float64) @ np.asarray(wv_a, np.float64)).astype(f)
    wc_t = (np.asarray(t2a_out_w, np.float64) @ np.asarray(wv_t, np.float64)).astype(f)
    bc_a = (np.asarray(bv_a, np.float64) @ np.asarray(a2t_out_w, np.float64).T
            + np.asarray(a2t_out_b, np.float64)).astype(f)
    bc_t = (np.asarray(bv_t, np.float64) @ np.asarray(t2a_out_w, np.float64).T
            + np.asarray(t2a_out_b, np.float64)).astype(f)

    generic_gb = not (
        np.all(np.asarray(ln1_g) == 1.0) and np.all(np.asarray(ln1_b) == 0.0)
        and np.all(np.asarray(ln2_g) == 1.0) and np.all(np.asarray(ln2_b) == 0.0)
    )
    generic_bc = not (np.all(bc_a == 0.0) and np.all(bc_t == 0.0))

    audio = np.ascontiguousarray(audio_embed, dtype=f)
    text = np.ascontiguousarray(text_embed, dtype=f)
    perm = _make_perm()

    from concurrent.futures import ThreadPoolExecutor

    def shard_xT(c):
        out = np.empty((2, E, ROWS), bf)
        base = c * ROWS
        out[0] = audio[base + perm].T.astype(bf)
        out[1] = text[base + perm].T.astype(bf)
        return out

    with ThreadPoolExecutor(max_workers=8) as ex:
        xTs = list(ex.map(shard_xT, range(N_CORES)))

    # w[mod] = Wc[mod].T (feat_in, feat_out)
    w_all = np.stack([wc_a.T, wc_t.T]).astype(bf)
    in_maps = []
    for c in range(N_CORES):
        m = {"xT": xTs[c], "w": w_all}
        if generic_bc:
            m["bc"] = np.stack([
                bc_a.reshape(1, E), bc_t.reshape(1, E),
            ]).astype(bf)
        if generic_gb:
            m["g"] = np.stack([
                np.asarray(ln1_g, f).reshape(1, E),
                np.asarray(ln2_g, f).reshape(1, E),
            ])
            m["b"] = np.stack([
                np.asarray(ln1_b, f).reshape(1, E),
                np.asarray(ln2_b, f).reshape(1, E),
            ])
        in_maps.append(m)
    return in_maps, generic_gb, generic_bc


def _run(in_maps, generic_gb, generic_bc, trace=False):
    import sys
    if "/opt/trn_rl_repo" not in sys.path:
        sys.path.insert(0, "/opt/trn_rl_repo")
    from concourse.bass_utils import run_bass_kernel_spmd

    nc = _get_program(generic_gb, generic_bc)
    res = run_bass_kernel_spmd(
        nc, in_maps, list(range(N_CORES)), trace=trace,
    )
    return res


def kernel(**inputs):
    import sys
    if "/opt/trn_rl_repo" not in sys.path:
        sys.path.insert(0, "/opt/trn_rl_repo")
    in_maps, generic_gb, generic_bc = _prep_host(**inputs)
    res = _run(in_maps, generic_gb, generic_bc,
               trace=bool(os.environ.get("KERNEL_TRACE")))
    f = np.float32
    audio_out = np.concatenate(
        [r["y"][:, 0, :].astype(f) for r in res.results], axis=0)
    text_out = np.concatenate(
        [r["y"][:, 1, :].astype(f) for r in res.results], axis=0)
    kernel.last_exec_time_ns = res.exec_time_ns
    kernel.last_results = res
    return (audio_out, text_out)
